# revision 42
# baseline (speedup 1.0000x reference)
"""ArcFace (AngularPenaltySMLoss) distributed Bass kernel for 8 TRN2 NeuronCores.

Strategy (vocab/tensor parallel, transposed layout):
  - W [50000, 512] sharded along classes: core k owns classes
    [6250k, 6250(k+1)), padded to 6272 = 49*128 (pad class -> logit 0 ->
    exp = 1.0, subtracted as a constant on the host).
  - Host L2-normalizes x once and ships xn^T in fp8 K-pair layout; W shard
    likewise. All scales fold into one constant Exp scale, so the device
    needs no norms, no per-row scale APs, and no Sqrt table loads.
  - Device computes logits TRANSPOSED: psum tile [128 class-partitions,
    1024 row-columns] = wt.T @ xnt via fp8 DoubleRow matmuls (2 K-pair
    passes x 2 chunks of 512), 4 psum tiles in flight.
  - Exp runs on TWO engines: ScalarE drains ~5/6 of the tiles with a
    constant-scale Exp (bf16 out); the idle VectorE drains the rest with a
    Schraudolph-style bit-trick exp: bits16 = round(logit*A + C) placed so
    that viewing the int16 as bf16 yields ~exp(S*l) (2.5% RMS, mean-tuned
    to <1e-4 bias; tolerance is 2e-2). That keeps ScalarE's 1 elem/lane/
    cycle Exp from being the critical path - the fp8 DoubleRow matmul is.
  - Per-row sums over the 6272 local classes = partition-axis sums:
    VectorE accumulates the 49 exp'd class-tiles elementwise (bf16 2x
    mode), then a ones-vector matmul on TensorE collapses the 128
    partitions into out[1, rows]; Scalar/Vector split the psum->SBUF
    copies and one DMA returns [1, 4096] per-core row-sums.
  - Host: all-reduce the 8 row-sum vectors, subtract the 176 pad ones,
    compute exact f64 target dots t_n = xn[n].W[target_n], and finish
      num = S*cos(acos(t)+m);  L = num - log(exp(num) + rowsum - pads
                                              - exp(S*t));  out = -mean(L)
"""

import functools
import math
import sys

import numpy as np

sys.path.insert(0, "/opt/trn_rl_repo")

N, D, C = 4096, 512, 50000
NCORES = 8
CSH = C // NCORES          # 6250 classes per core
CPAD = 6272                # 49*128
NCT = CPAD // 128          # 49 class tiles
S = 30.0
MARG = 0.4
EPS = 1e-7
SCALE_W = 512.0            # fp8 scale for W
SCALE_X = 64.0             # fp8 scale for normalized x
ESC = S / (SCALE_X * SCALE_W)   # constant Exp scale on ScalarE
PADS_TOTAL = float((CPAD - CSH) * NCORES)   # 176 pad classes, each exp(0)=1
RC = 1024                  # row chunk = psum tile free dim (2 banks)
NRC = N // RC              # 4
# Schraudolph bit-exp constants (VectorE offload): bits16 = l*A + C, viewed
# as bf16 == ~exp(S * l / (SCALE_X*SCALE_W)). tune=-0.0575 zeroes the mean
# relative error for the uniform-mantissa limit (validated in numpy).
LOG2E = 1.4426950408889634
A_MUL = S * LOG2E / (SCALE_X * SCALE_W) * 128.0
C_ADD = (127.0 - 0.0575) * 128.0


def _dve_tile(gi):
    """Which (rc*NCT+ct) tiles VectorE exps (~35/196), never a ct==0 tile.
    None in the last 6: at the end of the stream ACT drains the exp backlog
    (1.11us/tile) while DVE only chases the adds - shortest tail chain."""
    if gi >= 190:
        return False
    return gi % NCT != 0 and gi % 11 in (2, 7)


def build_graph():
    from concourse import bacc, bass, mybir, tile

    f32 = mybir.dt.float32
    bf16 = mybir.dt.bfloat16
    i16 = mybir.dt.int16
    f8 = mybir.dt.float8e4
    AF = mybir.ActivationFunctionType
    ALU = mybir.AluOpType

    nc = bacc.Bacc(
        "TRN2",
        target_bir_lowering=False,
        debug=False,
        enable_asserts=False,
        num_devices=NCORES,
    )

    xt_d = nc.dram_tensor("xt", [2, 128, 2, N], f8, kind="ExternalInput")
    wt_d = nc.dram_tensor("wt", [2, 128, 2, CPAD], f8, kind="ExternalInput")
    on_d = nc.dram_tensor("ones", [128, 1], bf16, kind="ExternalInput")
    out_d = nc.dram_tensor("out", [1, N], f32, kind="ExternalOutput")

    with tile.TileContext(nc) as tc:
        with (
            tc.tile_pool(name="big", bufs=1) as bigp,
            tc.tile_pool(name="es", bufs=6) as esp,
            tc.tile_pool(name="bits", bufs=3) as btp,
            tc.tile_pool(name="ps", bufs=4, space="PSUM") as pp,
        ):
            wt_sb = [
                bigp.tile([128, 2, CPAD], f8, name=f"wtsb{g}", tag=f"wtsb{g}")
                for g in range(2)
            ]
            xt_sb = [
                bigp.tile([128, 2, N], f8, name=f"xtsb{g}", tag=f"xtsb{g}")
                for g in range(2)
            ]
            ones_sb = bigp.tile([128, 1], bf16, name="ones_sb")
            acc = [
                [
                    bigp.tile([128, RC], bf16, name=f"acc{r}_{p}", tag=f"acc{r}_{p}")
                    for p in range(2)
                ]
                for r in range(NRC)
            ]
            CONT = bigp.tile([1, N], f32, name="CONT")

            # ---------- DMA ----------
            # the 4 transfers gating the first matmul go out on 4 different
            # engines' queues in parallel (descriptor issue is ~0.7us each,
            # serial per engine); everything else streams on sync/gpsimd in
            # consumption order.
            # both inputs of the very first matmul (wt[g0] ct0, xt[g0]) at
            # the HEAD of their queues; g2=1's operands (needed 4 MMs later)
            # ride gpsimd
            nc.sync.dma_start(wt_sb[0][:, :, 0:128], wt_d.ap()[0][:, :, 0:128])
            nc.scalar.dma_start(xt_sb[0][:, :, 0:512], xt_d.ap()[0][:, :, 0:512])
            nc.gpsimd.dma_start(xt_sb[1][:, :, 0:512], xt_d.ap()[1][:, :, 0:512])
            nc.gpsimd.dma_start(wt_sb[1][:, :, 0:128], wt_d.ap()[1][:, :, 0:128])
            # first wt chunks ride the scalar queue so they aren't stuck
            # behind xt transfers; sync then carries a pure 512-col wt
            # stream and the rc1 xt chunk; gpsimd takes the rest of xt
            for c0, c1 in ((128, 384), (384, 640)):
                for g in range(2):
                    nc.scalar.dma_start(
                        wt_sb[g][:, :, c0:c1], wt_d.ap()[g][:, :, c0:c1]
                    )
            for g in range(2):
                nc.gpsimd.dma_start(
                    xt_sb[g][:, :, 512:RC], xt_d.ap()[g][:, :, 512:RC]
                )
            # the last quarter of W rides gpsimd (idle after the early xt
            # chunks) - relieves the sync queue in its tightest window
            wchunks = [(c, min(c + 512, CPAD)) for c in range(640, 4736, 512)]
            for c0, c1 in wchunks:
                for g in range(2):
                    nc.sync.dma_start(
                        wt_sb[g][:, :, c0:c1], wt_d.ap()[g][:, :, c0:c1]
                    )
            for g in range(2):
                nc.sync.dma_start(
                    xt_sb[g][:, :, RC:2 * RC], xt_d.ap()[g][:, :, RC:2 * RC]
                )
            nc.gpsimd.dma_start(ones_sb[:], on_d.ap()[:, :])
            for c0, c1 in [(c, min(c + 512, CPAD)) for c in range(4736, CPAD, 512)]:
                for g in range(2):
                    nc.gpsimd.dma_start(
                        wt_sb[g][:, :, c0:c1], wt_d.ap()[g][:, :, c0:c1]
                    )
            for r in range(2, NRC):
                for g in range(2):
                    nc.gpsimd.dma_start(
                        xt_sb[g][:, :, r * RC:(r + 1) * RC],
                        xt_d.ap()[g][:, :, r * RC:(r + 1) * RC],
                    )

            # ---------- HAM warm-up ----------
            # The PE is data-starved for the first ~12us, so its clock gate
            # sits at K=4/8 (1.2 GHz) and the first ~8 real matmuls run at
            # half speed. Garbage matmuls on a memset tile (never read; the
            # psum slots are overwritten with start=True later) keep the
            # activity window hot so the real stream begins at 2.4 GHz.
            warm = bigp.tile([128, 512], bf16, name="warm")
            nc.vector.memset(warm[:], 0.0)
            for w in range(14):
                wp = pp.tile([128, RC], f32, name="wp", tag="pg")
                nc.tensor.matmul(
                    out=wp[0:1, 0:512],
                    lhsT=warm[:, 0:1],
                    rhs=warm[:],
                    start=True,
                    stop=True,
                )

            # ---------- main loop: matmul + two-engine exp + accumulate ----
            vfin = [0] * NRC   # which acc[rc][...] holds the final sum
            for rc in range(NRC):
                vidx = 0   # chain adds so far
                for ct in range(NCT):
                    gi = rc * NCT + ct
                    pg = pp.tile([128, RC], f32, name="pg", tag="pg")
                    for g2 in range(2):
                        for cc in range(RC // 512):
                            c0 = rc * RC + cc * 512
                            nc.tensor.matmul(
                                out=pg[:, cc * 512:(cc + 1) * 512],
                                lhsT=wt_sb[g2][:, :, ct * 128:(ct + 1) * 128],
                                rhs=xt_sb[g2][:, :, c0:c0 + 512],
                                start=(g2 == 0),
                                stop=(g2 == 1),
                                perf_mode=mybir.MatmulPerfMode.DoubleRow,
                            )
                    if ct == 0:
                        nc.scalar.activation(
                            out=acc[rc][0][:], in_=pg[:], func=AF.Exp, scale=ESC
                        )
                    elif _dve_tile(gi):
                        bits = btp.tile([128, RC], i16, name="bits", tag="bits")
                        nc.vector.tensor_scalar(
                            bits[:], pg[:], A_MUL, C_ADD, ALU.mult, ALU.add
                        )
                        nc.vector.tensor_add(
                            acc[rc][(vidx + 1) % 2][:],
                            acc[rc][vidx % 2][:],
                            bits[:].bitcast(bf16),
                        )
                        vidx += 1
                    else:
                        es = esp.tile([128, RC], bf16, name="es", tag="es")
                        nc.scalar.activation(
                            out=es[:], in_=pg[:], func=AF.Exp, scale=ESC
                        )
                        nc.vector.tensor_add(
                            acc[rc][(vidx + 1) % 2][:],
                            acc[rc][vidx % 2][:],
                            es[:],
                        )
                        vidx += 1
                vfin[rc] = vidx % 2

            # ---------- partition-axis reduce via ones matmul ----------
            opss = []
            for rc in range(NRC):
                ops = pp.tile([128, RC], f32, name="ops", tag="pg")
                for cc in range(RC // 512):
                    nc.tensor.matmul(
                        out=ops[0:1, cc * 512:(cc + 1) * 512],
                        lhsT=ones_sb[:, 0:1],
                        rhs=acc[rc][vfin[rc]][:, cc * 512:(cc + 1) * 512],
                        start=True,
                        stop=True,
                    )
                opss.append(ops)
            # rc0-2 copies go to ScalarE only (it idles at the end while
            # VectorE still chases the last add chain); rc3 splits across
            # both once DVE is finally free
            for rc in range(NRC):
                h = RC // 2
                if rc < NRC - 1:
                    nc.scalar.copy(
                        CONT[0:1, rc * RC:(rc + 1) * RC], opss[rc][0:1, 0:RC]
                    )
                else:
                    nc.scalar.copy(
                        CONT[0:1, rc * RC:rc * RC + h], opss[rc][0:1, 0:h]
                    )
                    nc.vector.tensor_copy(
                        CONT[0:1, rc * RC + h:(rc + 1) * RC], opss[rc][0:1, h:RC]
                    )
                    # half-DMAs: the scalar half flies without waiting for
                    # the vector half's copy; the final DMA is 2KB
                    nc.sync.dma_start(
                        out_d.ap()[:, rc * RC:rc * RC + h],
                        CONT[0:1, rc * RC:rc * RC + h],
                    )
                    nc.sync.dma_start(
                        out_d.ap()[:, rc * RC + h:(rc + 1) * RC],
                        CONT[0:1, rc * RC + h:(rc + 1) * RC],
                    )
                    continue
                # per-chunk output DMA: earlier chunks fly while the last
                # one is still being copied
                nc.sync.dma_start(
                    out_d.ap()[:, rc * RC:(rc + 1) * RC],
                    CONT[0:1, rc * RC:(rc + 1) * RC],
                )

    nc.compile()
    return nc


@functools.lru_cache(maxsize=1)
def _compiled():
    return build_graph()


def _prep(x, W, target):
    import ml_dtypes

    f8 = ml_dtypes.float8_e4m3fn
    x = np.asarray(x, dtype=np.float32)
    W = np.asarray(W, dtype=np.float32)
    target = np.asarray(target, dtype=np.int32)

    xn = x / np.linalg.norm(x, axis=1, keepdims=True)
    # xt[g, p, i, n] = xn[n, (2g+i)*128 + p] * SCALE_X  (fp8 K-pairs)
    xt = np.ascontiguousarray(
        np.clip((xn.T * SCALE_X).reshape(2, 2, 128, N).transpose(0, 2, 1, 3),
                -240, 240)
    ).astype(f8)
    ones = np.ones((128, 1), dtype=ml_dtypes.bfloat16)
    in_maps = []
    for k in range(NCORES):
        wtp = np.zeros((D, CPAD), dtype=np.float32)
        wtp[:, :CSH] = W[k * CSH:(k + 1) * CSH].T * SCALE_W
        wt = np.ascontiguousarray(
            np.clip(wtp.reshape(2, 2, 128, CPAD).transpose(0, 2, 1, 3), -240, 240)
        ).astype(f8)
        in_maps.append({"xt": xt, "wt": wt, "ones": ones})

    # exact target-cosine dots on the host (f64)
    tg = np.einsum(
        "nd,nd->n",
        xn.astype(np.float64),
        W[target].astype(np.float64),
    )
    return in_maps, tg


def _combine(parts, tg):
    """Host all-reduce of per-core [1, 4096] row-sums + scalar tail."""
    rowsum = np.zeros(N, dtype=np.float64)
    for p in parts:
        rowsum += np.asarray(p, dtype=np.float64).reshape(N)
    tcl = np.clip(tg, -1.0 + EPS, 1.0 - EPS)
    num = S * (tcl * math.cos(MARG) - np.sqrt(1.0 - tcl * tcl) * math.sin(MARG))
    excl = rowsum - PADS_TOTAL - np.exp(S * tg)
    denom = np.exp(num) + excl
    L = num - np.log(denom)
    return np.float32(-np.mean(L))


def kernel_run(x, W, target, trace=False, **kw):
    """Returns (loss_scalar, BassKernelResults)."""
    from concourse import bass_utils

    nc = _compiled()
    in_maps, tg = _prep(x, W, target)
    res = bass_utils.run_bass_kernel_spmd(
        nc, in_maps, core_ids=list(range(NCORES)), trace=trace, **kw
    )
    loss = _combine([r["out"] for r in res.results], tg)
    return np.asarray(loss, dtype=np.float32), res


def kernel(x, W, target):
    loss, _ = kernel_run(x, W, target, trace=False)
    return loss


if __name__ == "__main__":
    nc = build_graph()
    print("graph built + compiled OK")


# revision 43
# speedup vs baseline: 1.0070x; 1.0070x over previous
"""ArcFace (AngularPenaltySMLoss) distributed Bass kernel for 8 TRN2 NeuronCores.

Strategy (vocab/tensor parallel, transposed layout):
  - W [50000, 512] sharded along classes: core k owns classes
    [6250k, 6250(k+1)), padded to 6272 = 49*128 (pad class -> logit 0 ->
    exp = 1.0, subtracted as a constant on the host).
  - Host L2-normalizes x once and ships xn^T in fp8 K-pair layout; W shard
    likewise. All scales fold into one constant Exp scale, so the device
    needs no norms, no per-row scale APs, and no Sqrt table loads.
  - Device computes logits TRANSPOSED: psum tile [128 class-partitions,
    1024 row-columns] = wt.T @ xnt via fp8 DoubleRow matmuls (2 K-pair
    passes x 2 chunks of 512), 4 psum tiles in flight.
  - Exp runs on TWO engines: ScalarE drains ~5/6 of the tiles with a
    constant-scale Exp (bf16 out); the idle VectorE drains the rest with a
    Schraudolph-style bit-trick exp: bits16 = round(logit*A + C) placed so
    that viewing the int16 as bf16 yields ~exp(S*l) (2.5% RMS, mean-tuned
    to <1e-4 bias; tolerance is 2e-2). That keeps ScalarE's 1 elem/lane/
    cycle Exp from being the critical path - the fp8 DoubleRow matmul is.
  - Per-row sums over the 6272 local classes = partition-axis sums:
    VectorE accumulates the 49 exp'd class-tiles elementwise (bf16 2x
    mode), then a ones-vector matmul on TensorE collapses the 128
    partitions into out[1, rows]; Scalar/Vector split the psum->SBUF
    copies and one DMA returns [1, 4096] per-core row-sums.
  - Host: all-reduce the 8 row-sum vectors, subtract the 176 pad ones,
    compute exact f64 target dots t_n = xn[n].W[target_n], and finish
      num = S*cos(acos(t)+m);  L = num - log(exp(num) + rowsum - pads
                                              - exp(S*t));  out = -mean(L)
"""

import functools
import math
import sys

import numpy as np

sys.path.insert(0, "/opt/trn_rl_repo")

N, D, C = 4096, 512, 50000
NCORES = 8
CSH = C // NCORES          # 6250 classes per core
CPAD = 6272                # 49*128
NCT = CPAD // 128          # 49 class tiles
S = 30.0
MARG = 0.4
EPS = 1e-7
SCALE_W = 512.0            # fp8 scale for W
SCALE_X = 64.0             # fp8 scale for normalized x
ESC = S / (SCALE_X * SCALE_W)   # constant Exp scale on ScalarE
PADS_TOTAL = float((CPAD - CSH) * NCORES)   # 176 pad classes, each exp(0)=1
RC = 1024                  # row chunk = psum tile free dim (2 banks)
NRC = N // RC              # 4
# Schraudolph bit-exp constants (VectorE offload): bits16 = l*A + C, viewed
# as bf16 == ~exp(S * l / (SCALE_X*SCALE_W)). tune=-0.0575 zeroes the mean
# relative error for the uniform-mantissa limit (validated in numpy).
LOG2E = 1.4426950408889634
A_MUL = S * LOG2E / (SCALE_X * SCALE_W) * 128.0
C_ADD = (127.0 - 0.0575) * 128.0


def _dve_tile(gi):
    """Which (rc*NCT+ct) tiles VectorE exps (~35/196), never a ct==0 tile.
    None in the last 6: at the end of the stream ACT drains the exp backlog
    (1.11us/tile) while DVE only chases the adds - shortest tail chain."""
    if gi >= 190:
        return False
    return gi % NCT != 0 and gi % 11 in (2, 7)


def build_graph():
    from concourse import bacc, bass, mybir, tile

    f32 = mybir.dt.float32
    bf16 = mybir.dt.bfloat16
    i16 = mybir.dt.int16
    f8 = mybir.dt.float8e4
    AF = mybir.ActivationFunctionType
    ALU = mybir.AluOpType

    nc = bacc.Bacc(
        "TRN2",
        target_bir_lowering=False,
        debug=False,
        enable_asserts=False,
        num_devices=NCORES,
    )

    xt_d = nc.dram_tensor("xt", [2, 128, 2, N], f8, kind="ExternalInput")
    wt_d = nc.dram_tensor("wt", [2, 128, 2, CPAD], f8, kind="ExternalInput")
    on_d = nc.dram_tensor("ones", [128, 1], bf16, kind="ExternalInput")
    out_d = nc.dram_tensor("out", [1, N], f32, kind="ExternalOutput")

    with tile.TileContext(nc) as tc:
        with (
            tc.tile_pool(name="big", bufs=1) as bigp,
            tc.tile_pool(name="es", bufs=6) as esp,
            tc.tile_pool(name="bits", bufs=3) as btp,
            tc.tile_pool(name="ps", bufs=4, space="PSUM") as pp,
        ):
            wt_sb = [
                bigp.tile([128, 2, CPAD], f8, name=f"wtsb{g}", tag=f"wtsb{g}")
                for g in range(2)
            ]
            xt_sb = [
                bigp.tile([128, 2, N], f8, name=f"xtsb{g}", tag=f"xtsb{g}")
                for g in range(2)
            ]
            ones_sb = bigp.tile([128, 1], bf16, name="ones_sb")
            acc = [
                [
                    bigp.tile([128, RC], bf16, name=f"acc{r}_{p}", tag=f"acc{r}_{p}")
                    for p in range(2)
                ]
                for r in range(NRC)
            ]
            CONT = bigp.tile([1, N], f32, name="CONT")

            # ---------- DMA ----------
            # the 4 transfers gating the first matmul go out on 4 different
            # engines' queues in parallel (descriptor issue is ~0.7us each,
            # serial per engine); everything else streams on sync/gpsimd in
            # consumption order.
            nc.sync.dma_start(wt_sb[0][:, :, 0:128], wt_d.ap()[0][:, :, 0:128])
            nc.scalar.dma_start(wt_sb[1][:, :, 0:128], wt_d.ap()[1][:, :, 0:128])
            nc.scalar.dma_start(xt_sb[0][:, :, 0:512], xt_d.ap()[0][:, :, 0:512])
            nc.gpsimd.dma_start(xt_sb[1][:, :, 0:512], xt_d.ap()[1][:, :, 0:512])
            # first wt chunks ride the scalar queue so they aren't stuck
            # behind xt transfers; sync then carries a pure 512-col wt
            # stream and the rc1 xt chunk; gpsimd takes the rest of xt
            for c0, c1 in ((128, 384), (384, 640)):
                for g in range(2):
                    nc.scalar.dma_start(
                        wt_sb[g][:, :, c0:c1], wt_d.ap()[g][:, :, c0:c1]
                    )
            for g in range(2):
                nc.gpsimd.dma_start(
                    xt_sb[g][:, :, 512:RC], xt_d.ap()[g][:, :, 512:RC]
                )
            # the last quarter of W rides gpsimd (idle after the early xt
            # chunks) - relieves the sync queue in its tightest window
            wchunks = [(c, min(c + 512, CPAD)) for c in range(640, 4736, 512)]
            for c0, c1 in wchunks:
                for g in range(2):
                    nc.sync.dma_start(
                        wt_sb[g][:, :, c0:c1], wt_d.ap()[g][:, :, c0:c1]
                    )
            for g in range(2):
                nc.sync.dma_start(
                    xt_sb[g][:, :, RC:2 * RC], xt_d.ap()[g][:, :, RC:2 * RC]
                )
            nc.gpsimd.dma_start(ones_sb[:], on_d.ap()[:, :])
            for c0, c1 in [(c, min(c + 512, CPAD)) for c in range(4736, CPAD, 512)]:
                for g in range(2):
                    nc.gpsimd.dma_start(
                        wt_sb[g][:, :, c0:c1], wt_d.ap()[g][:, :, c0:c1]
                    )
            for r in range(2, NRC):
                for g in range(2):
                    nc.gpsimd.dma_start(
                        xt_sb[g][:, :, r * RC:(r + 1) * RC],
                        xt_d.ap()[g][:, :, r * RC:(r + 1) * RC],
                    )

            # ---------- HAM warm-up ----------
            # The PE is data-starved for the first ~12us, so its clock gate
            # sits at K=4/8 (1.2 GHz) and the first ~8 real matmuls run at
            # half speed. Garbage matmuls on a memset tile (never read; the
            # psum slots are overwritten with start=True later) keep the
            # activity window hot so the real stream begins at 2.4 GHz.
            warm = bigp.tile([128, 512], bf16, name="warm")
            nc.vector.memset(warm[:], 0.0)
            for w in range(14):
                wp = pp.tile([128, RC], f32, name="wp", tag="pg")
                nc.tensor.matmul(
                    out=wp[0:1, 0:512],
                    lhsT=warm[:, 0:1],
                    rhs=warm[:],
                    start=True,
                    stop=True,
                )

            # ---------- main loop: matmul + two-engine exp + accumulate ----
            vfin = [0] * NRC   # which acc[rc][...] holds the final sum
            for rc in range(NRC):
                vidx = 0   # chain adds so far
                for ct in range(NCT):
                    gi = rc * NCT + ct
                    pg = pp.tile([128, RC], f32, name="pg", tag="pg")
                    for g2 in range(2):
                        for cc in range(RC // 512):
                            c0 = rc * RC + cc * 512
                            nc.tensor.matmul(
                                out=pg[:, cc * 512:(cc + 1) * 512],
                                lhsT=wt_sb[g2][:, :, ct * 128:(ct + 1) * 128],
                                rhs=xt_sb[g2][:, :, c0:c0 + 512],
                                start=(g2 == 0),
                                stop=(g2 == 1),
                                perf_mode=mybir.MatmulPerfMode.DoubleRow,
                            )
                    if ct == 0:
                        nc.scalar.activation(
                            out=acc[rc][0][:], in_=pg[:], func=AF.Exp, scale=ESC
                        )
                    elif _dve_tile(gi):
                        bits = btp.tile([128, RC], i16, name="bits", tag="bits")
                        nc.vector.tensor_scalar(
                            bits[:], pg[:], A_MUL, C_ADD, ALU.mult, ALU.add
                        )
                        nc.vector.tensor_add(
                            acc[rc][(vidx + 1) % 2][:],
                            acc[rc][vidx % 2][:],
                            bits[:].bitcast(bf16),
                        )
                        vidx += 1
                    else:
                        es = esp.tile([128, RC], bf16, name="es", tag="es")
                        nc.scalar.activation(
                            out=es[:], in_=pg[:], func=AF.Exp, scale=ESC
                        )
                        nc.vector.tensor_add(
                            acc[rc][(vidx + 1) % 2][:],
                            acc[rc][vidx % 2][:],
                            es[:],
                        )
                        vidx += 1
                vfin[rc] = vidx % 2

            # ---------- partition-axis reduce via ones matmul ----------
            opss = []
            for rc in range(NRC):
                ops = pp.tile([128, RC], f32, name="ops", tag="pg")
                for cc in range(RC // 512):
                    nc.tensor.matmul(
                        out=ops[0:1, cc * 512:(cc + 1) * 512],
                        lhsT=ones_sb[:, 0:1],
                        rhs=acc[rc][vfin[rc]][:, cc * 512:(cc + 1) * 512],
                        start=True,
                        stop=True,
                    )
                opss.append(ops)
            # rc0-2 copies go to ScalarE only (it idles at the end while
            # VectorE still chases the last add chain); rc3 splits across
            # both once DVE is finally free
            for rc in range(NRC):
                h = RC // 2
                if rc < NRC - 1:
                    nc.scalar.copy(
                        CONT[0:1, rc * RC:(rc + 1) * RC], opss[rc][0:1, 0:RC]
                    )
                else:
                    nc.scalar.copy(
                        CONT[0:1, rc * RC:rc * RC + h], opss[rc][0:1, 0:h]
                    )
                    nc.vector.tensor_copy(
                        CONT[0:1, rc * RC + h:(rc + 1) * RC], opss[rc][0:1, h:RC]
                    )
                    # half-DMAs: the scalar half flies without waiting for
                    # the vector half's copy; the final DMA is 2KB
                    nc.sync.dma_start(
                        out_d.ap()[:, rc * RC:rc * RC + h],
                        CONT[0:1, rc * RC:rc * RC + h],
                    )
                    nc.sync.dma_start(
                        out_d.ap()[:, rc * RC + h:(rc + 1) * RC],
                        CONT[0:1, rc * RC + h:(rc + 1) * RC],
                    )
                    continue
                # per-chunk output DMA: earlier chunks fly while the last
                # one is still being copied
                nc.sync.dma_start(
                    out_d.ap()[:, rc * RC:(rc + 1) * RC],
                    CONT[0:1, rc * RC:(rc + 1) * RC],
                )

    nc.compile()
    return nc


@functools.lru_cache(maxsize=1)
def _compiled():
    return build_graph()


def _prep(x, W, target):
    import ml_dtypes

    f8 = ml_dtypes.float8_e4m3fn
    x = np.asarray(x, dtype=np.float32)
    W = np.asarray(W, dtype=np.float32)
    target = np.asarray(target, dtype=np.int32)

    xn = x / np.linalg.norm(x, axis=1, keepdims=True)
    # xt[g, p, i, n] = xn[n, (2g+i)*128 + p] * SCALE_X  (fp8 K-pairs)
    xt = np.ascontiguousarray(
        np.clip((xn.T * SCALE_X).reshape(2, 2, 128, N).transpose(0, 2, 1, 3),
                -240, 240)
    ).astype(f8)
    ones = np.ones((128, 1), dtype=ml_dtypes.bfloat16)
    in_maps = []
    for k in range(NCORES):
        wtp = np.zeros((D, CPAD), dtype=np.float32)
        wtp[:, :CSH] = W[k * CSH:(k + 1) * CSH].T * SCALE_W
        wt = np.ascontiguousarray(
            np.clip(wtp.reshape(2, 2, 128, CPAD).transpose(0, 2, 1, 3), -240, 240)
        ).astype(f8)
        in_maps.append({"xt": xt, "wt": wt, "ones": ones})

    # exact target-cosine dots on the host (f64)
    tg = np.einsum(
        "nd,nd->n",
        xn.astype(np.float64),
        W[target].astype(np.float64),
    )
    return in_maps, tg


def _combine(parts, tg):
    """Host all-reduce of per-core [1, 4096] row-sums + scalar tail."""
    rowsum = np.zeros(N, dtype=np.float64)
    for p in parts:
        rowsum += np.asarray(p, dtype=np.float64).reshape(N)
    tcl = np.clip(tg, -1.0 + EPS, 1.0 - EPS)
    num = S * (tcl * math.cos(MARG) - np.sqrt(1.0 - tcl * tcl) * math.sin(MARG))
    excl = rowsum - PADS_TOTAL - np.exp(S * tg)
    denom = np.exp(num) + excl
    L = num - np.log(denom)
    return np.float32(-np.mean(L))


def kernel_run(x, W, target, trace=False, **kw):
    """Returns (loss_scalar, BassKernelResults)."""
    from concourse import bass_utils

    nc = _compiled()
    in_maps, tg = _prep(x, W, target)
    res = bass_utils.run_bass_kernel_spmd(
        nc, in_maps, core_ids=list(range(NCORES)), trace=trace, **kw
    )
    loss = _combine([r["out"] for r in res.results], tg)
    return np.asarray(loss, dtype=np.float32), res


def kernel(x, W, target):
    loss, _ = kernel_run(x, W, target, trace=False)
    return loss


if __name__ == "__main__":
    nc = build_graph()
    print("graph built + compiled OK")
